# revision 1
# baseline (speedup 1.0000x reference)
"""Trainium2 Bass kernel for AdaptiveLogSoftmaxWithLoss (moe_routing).

Sharding: the three class dimensions are zero-padded and tensor-sharded
across the 8 cores (head 4002->4096, tail0 16000->16384, tail1
30257->30720), so every core runs an identical SPMD program over all 2048
samples with 1/8 of the output classes (6400 columns).

Per core:
  - hidden projections h0T=[512,2048], h1T=[256,2048] in transposed layout
    (fp8 DoubleRow GEMMs, inp scaled 16x / w1 64x), cast to bf16 (for the
    target dots) and to fp8*8 (as lhsT of the tail GEMMs),
  - logit shards computed in [sample, class] PSUM groups up to 4 banks wide
    (fp8 DoubleRow; head also fp8), one ACT exp (+accum_out, descaled via
    the activation scale) per group -> partial per-row sum-exp.  Logits are
    small by construction (|x| < ~4) so no max subtraction is needed,
  - target logits: the head uses a fused DVE (iota==rel)*logit pass on its
    PSUM group; the tails dot bf16 natural-layout hidden rows (batched XBAR
    DMA transposes of hT) against host-gathered target weight rows that are
    zeroed on non-owner cores,
  - emission order interleaves head groups with hidden0 blocks and hidden1
    blocks into the tail0 loop so the scalar engine (the exp bottleneck,
    ~13M elements/core) stays fed while the PE runs GEMMs.

Host combine: sum partials over cores, subtract the exact exp(0)=1
contribution of the zero-padded columns, lse = log(sum), gathers sum to the
single owner value, then NLL = -(head + masked tail terms) as in the
reference.  All heavy math (GEMMs, exp, reductions, gathers) runs on
device; the host only shards, pads, quantizes, and combines [N]-vectors.
"""

import numpy as np
import ml_dtypes

import concourse.bass as bass
import concourse.bacc as bacc
import concourse.mybir as mybir
import concourse.tile as tile
from concourse.bass_utils import run_bass_kernel_spmd

BF16 = ml_dtypes.bfloat16
FP8 = ml_dtypes.float8_e4m3
H_SCALE = 8.0     # h cast to fp8 at 8x
W_SCALE = 64.0    # tail w2 cast to fp8 at 64x
IN_SCALE = 16.0   # inp cast to fp8 at 16x
W1_SCALE = 64.0   # w1 / head_w cast to fp8 at 64x
HID_DESCALE = 1.0 / (IN_SCALE * W1_SCALE)
NCORES = 8
N, D = 2048, 1024
H0, H1 = 512, 256
C0, C1 = 4000, 20000
HEAD = 4002        # 4000 shortlist + 2 cluster-logit columns
HEAD_PAD = 4096    # padded so 8 cores get 512 each
T0 = 16000
T0_PAD = 16000     # divides by 8 exactly (2000 each, no padding)
T1 = 30257
T1_PAD = 30720     # padded so 8 cores get 3840 each
WH, W0, W1 = HEAD_PAD // 8, T0_PAD // 8, T1_PAD // 8   # 512, 2000, 3840
MT = N // 128                                          # 16 sample tiles
PAD_H = HEAD_PAD - HEAD   # 94 zero columns, all on core 7
PAD_0 = T0_PAD - T0       # 384 zero columns, all on core 7
PAD_1 = T1_PAD - T1       # 463 zero columns, all on core 7

# module-level knobs for test.py (harness never touches these)
TRACE = False
LAST_RESULT = None

_CACHED_NC = None


def _build_nc():
    nc = bacc.Bacc(None)
    BF = mybir.dt.bfloat16
    F8 = mybir.dt.float8e4
    F32 = mybir.dt.float32
    AX = mybir.AxisListType
    OP = mybir.AluOpType
    ACTF = mybir.ActivationFunctionType

    inpT_d = nc.dram_tensor("inpT", [128, D // 128, N], F8, kind="ExternalInput")
    w1t0_d = nc.dram_tensor("w1t0", [128, D // 128, H0], F8, kind="ExternalInput")
    w1t1_d = nc.dram_tensor("w1t1", [128, D // 128, H1], F8, kind="ExternalInput")
    hwT_d = nc.dram_tensor("hwT", [128, D // 128, WH], F8, kind="ExternalInput")
    w2t0_d = nc.dram_tensor("w2t0", [128, H0 // 128, W0], F8, kind="ExternalInput")
    w2t1_d = nc.dram_tensor("w2t1", [128, H1 // 128, W1], F8, kind="ExternalInput")
    wg0_d = nc.dram_tensor("wg0", [128, MT, H0], BF, kind="ExternalInput")
    wg1_d = nc.dram_tensor("wg1", [128, MT, H1], BF, kind="ExternalInput")
    iota_d = nc.dram_tensor("iota", [128, WH], F32, kind="ExternalInput")
    rels_d = nc.dram_tensor("rels", [128, MT, 3], F32, kind="ExternalInput")
    res_d = nc.dram_tensor("res", [128, MT, 6], F32, kind="ExternalOutput")

    with tile.TileContext(nc) as tc:
        with (
            tc.tile_pool(name="const", bufs=1) as cp,
            tc.tile_pool(name="work", bufs=3) as wp,
            tc.tile_pool(name="parts", bufs=4) as pp,
        ):
            inpT = cp.tile([128, D // 128, N], F8)
            w1t0 = cp.tile([128, D // 128, H0], F8)
            w1t1 = cp.tile([128, D // 128, H1], F8)
            hwT = cp.tile([128, D // 128, WH], F8)
            w2t0 = cp.tile([128, H0 // 128, W0], F8)
            w2t1 = cp.tile([128, H1 // 128, W1], F8)
            wg0 = cp.tile([128, MT, H0], BF)
            wg1 = cp.tile([128, MT, H1], BF)
            iota = cp.tile([128, WH], F32)
            rels = cp.tile([128, MT, 3], F32)
            h0T = cp.tile([128, H0 // 128, N], BF)
            h1T = cp.tile([128, H1 // 128, N], BF)
            h0T8 = cp.tile([128, H0 // 128, N], F8)
            h1T8 = cp.tile([128, H1 // 128, N], F8)
            h0n = cp.tile([128, MT, H0], BF)
            h1n = cp.tile([128, MT, H1], BF)
            res = cp.tile([128, MT, 6], F32)

            # loads ordered to match emission: head first, then hidden
            for kt in range(D // 128):
                nc.sync.dma_start(inpT[:, kt], inpT_d[:, kt])
                nc.sync.dma_start(hwT[:, kt], hwT_d[:, kt])
            nc.sync.dma_start(iota[:], iota_d[:])
            nc.sync.dma_start(rels[:], rels_d[:])
            nc.sync.dma_start(w1t0[:], w1t0_d[:])
            nc.sync.dma_start(w1t1[:], w1t1_d[:])
            nc.sync.dma_start(w2t0[:], w2t0_d[:])
            nc.sync.dma_start(wg0[:], wg0_d[:])
            nc.sync.dma_start(w2t1[:], w2t1_d[:])
            nc.sync.dma_start(wg1[:], wg1_d[:])

            # Front phase (head + hidden0) uses 6 one-bank slots; the
            # mid/tail phases use 2 four-bank slots.  The pools are opened
            # sequentially (the phase boundary is already data-serialized
            # on h0T8, so the pool swap costs nothing).
            fpool_cm = tc.tile_pool(name="psumF", bufs=6, space="PSUM")
            fpool = fpool_cm.__enter__()
            psp = None

            def fslot(w):
                ps = fpool.tile([128, 512], F32, tag="front", name="ps")
                return ps[:, :w]

            def pslot(w):
                ps = psp.tile([128, 2048], F32, tag="logits", name="ps")
                return ps[:, :w]

            DESCALE = 1.0 / (H_SCALE * W_SCALE)
            DR = mybir.MatmulPerfMode.DoubleRow

            def hidden_block(hT, hT8, w1, hdim, mh, alloc):
                # one h k-tile: [128 h, 2048 samples] in 512-col chunks
                for rc in range(N // 512):
                    ps = alloc(512)
                    for kt in range(0, D // 128, 2):
                        nc.tensor.matmul(
                            ps[:],
                            w1[:, kt : kt + 2, mh * 128 : (mh + 1) * 128],
                            inpT[:, kt : kt + 2, rc * 512 : (rc + 1) * 512],
                            start=(kt == 0),
                            stop=(kt + 2 >= D // 128),
                            perf_mode=DR,
                        )
                    nc.vector.tensor_scalar_mul(
                        hT[:, mh, rc * 512 : (rc + 1) * 512], ps[:], HID_DESCALE
                    )
                    nc.vector.tensor_scalar_mul(
                        hT8[:, mh, rc * 512 : (rc + 1) * 512],
                        hT[:, mh, rc * 512 : (rc + 1) * 512],
                        H_SCALE,
                    )

            def head_group(m):
                ms = slice(m * 128, (m + 1) * 128)
                ps = fslot(WH)
                for kt in range(0, D // 128, 2):
                    nc.tensor.matmul(
                        ps[:],
                        inpT[:, kt : kt + 2, ms],
                        hwT[:, kt : kt + 2, :],
                        start=(kt == 0),
                        stop=(kt + 2 >= D // 128),
                        perf_mode=DR,
                    )
                sc_e = wp.tile([128, 2048], BF, tag="sc_e")
                nc.scalar.activation(
                    sc_e[:, :WH],
                    ps[:],
                    ACTF.Exp,
                    scale=HID_DESCALE,
                    accum_out=res[:, m, 0:1],
                )
                sc_t = wp.tile([128, WH], BF, tag="sc_t")
                nc.vector.scalar_tensor_tensor(
                    out=sc_t[:],
                    in0=iota[:],
                    scalar=rels[:, m, 0:1],
                    in1=ps[:],
                    op0=OP.is_equal,
                    op1=OP.mult,
                    accum_out=res[:, m, 3:4],
                )

            def tail_group(lhsT, w2, kdim, m, gw, goff, s_ap):
                # fp8 DoubleRow GEMM group + exp/accum partial sum
                ms = slice(m * 128, (m + 1) * 128)
                ps = pslot(gw)
                nsub = kdim // 128
                for co in range(0, gw, 512):
                    cw = min(512, gw - co)
                    for kt in range(0, nsub, 2):
                        nc.tensor.matmul(
                            ps[:, co : co + cw],
                            lhsT[:, kt : kt + 2, ms],
                            w2[:, kt : kt + 2, goff + co : goff + co + cw],
                            start=(kt == 0),
                            stop=(kt + 2 >= nsub),
                            perf_mode=DR,
                        )
                sc_e = wp.tile([128, 2048], BF, tag="sc_e")
                nc.scalar.activation(
                    sc_e[:, :gw], ps[:], ACTF.Exp, scale=DESCALE, accum_out=s_ap
                )

            def transposes(hT, hn, hdim):
                # batched XBAR transpose hT[h, r] -> hn[r, h]:
                # out[p, j, q] = in[q, j*128+p]
                for kt in range(hdim // 128):
                    nc.sync.dma_start_transpose(
                        hn[:, :, kt * 128 : (kt + 1) * 128], hT[:, kt, :]
                    )

            def dot(hn, wg, hdim, m, t_ap):
                sc_d = wp.tile([128, H0], BF, tag="sc_d")
                nc.vector.scalar_tensor_tensor(
                    out=sc_d[:, :hdim],
                    in0=hn[:, m, :],
                    scalar=1.0,
                    in1=wg[:, m, :],
                    op0=OP.mult,
                    op1=OP.mult,
                    accum_out=t_ap,
                )

            # emission order feeds ACT as early as possible:
            # head -> h0 hidden -> tail0 -> h1 hidden -> tail1
            with nc.named_scope("head_hidden0"):
                for i in range(H0 // 128):
                    for m in range(4 * i, 4 * i + 4):
                        head_group(m)
                    hidden_block(h0T, h0T8, w1t0, H0, i, fslot)
            fpool_cm.__exit__(None, None, None)
            psp_cm = tc.tile_pool(name="psum", bufs=2, space="PSUM")
            psp = psp_cm.__enter__()
            transposes(h0T, h0n, H0)
            with nc.named_scope("tail0_hidden1"):
                for m in range(MT):
                    tail_group(h0T8, w2t0, H0, m, W0, 0, res[:, m, 1:2])
                    dot(h0n, wg0, H0, m, res[:, m, 4:5])
                    if m in (6, 13):
                        hidden_block(h1T, h1T8, w1t1, H1, m == 13, pslot)
            transposes(h1T, h1n, H1)
            with nc.named_scope("tail1"):
                for m in range(MT):
                    spart = pp.tile([128, 2], F32, tag="spart")
                    dot(h1n, wg1, H1, m, res[:, m, 5:6])
                    # B group first: exp on ACT without accum, sum on DVE,
                    # so the final ACT exp (A group) overlaps the B reduce
                    ms = slice(m * 128, (m + 1) * 128)
                    ps = pslot(1792)
                    for co in range(0, 1792, 512):
                        cw = min(512, 1792 - co)
                        nc.tensor.matmul(
                            ps[:, co : co + cw],
                            h1T8[:, 0:2, ms],
                            w2t1[:, 0:2, 2048 + co : 2048 + co + cw],
                            start=True,
                            stop=True,
                            perf_mode=DR,
                        )
                    sc_e = wp.tile([128, 2048], BF, tag="sc_e")
                    nc.scalar.activation(
                        sc_e[:, :1792], ps[:], ACTF.Exp, scale=DESCALE
                    )
                    nc.vector.reduce_sum(spart[:, 1:2], sc_e[:, :1792], axis=AX.X)
                    tail_group(h1T8, w2t1, H1, m, 2048, 0, spart[:, 0:1])
                    nc.vector.reduce_sum(res[:, m, 2:3], spart[:], axis=AX.X)

            psp_cm.__exit__(None, None, None)
            nc.sync.dma_start(res_d[:], res[:])

    nc.finalize()
    return nc


def _get_nc():
    global _CACHED_NC
    if _CACHED_NC is None:
        _CACHED_NC = _build_nc()
    return _CACHED_NC


def _tiled(a2d):
    """[K, F] (K multiple of 128) -> contiguous [128, K//128, F]."""
    K, F = a2d.shape
    return np.ascontiguousarray(
        a2d.reshape(K // 128, 128, F).transpose(1, 0, 2)
    )


def _pm(vec):
    """[N] -> [128, MT] with [p, m] = vec[m*128+p]."""
    return np.ascontiguousarray(vec.reshape(MT, 128).T)


def _unpm(a):
    """[128, MT] -> [N]."""
    return np.ascontiguousarray(a.T).reshape(N)


def make_in_maps(inp, tgt, head_w, t0_w1, t0_w2, t1_w1, t1_w2):
    inp = np.asarray(inp, dtype=np.float32)
    tgt = np.asarray(tgt).astype(np.int64)

    inpT = _tiled((inp.T * IN_SCALE).astype(FP8))
    w1t0 = _tiled((np.asarray(t0_w1, np.float32).T * W1_SCALE).astype(FP8))
    w1t1 = _tiled((np.asarray(t1_w1, np.float32).T * W1_SCALE).astype(FP8))

    hwT_full = np.zeros((D, HEAD_PAD), FP8)
    hwT_full[:, :HEAD] = (np.asarray(head_w, np.float32).T * W1_SCALE).astype(FP8)
    w2t0_full = (np.asarray(t0_w2, np.float32).T * W_SCALE).astype(FP8)
    w2t1_full = np.zeros((H1, T1_PAD), FP8)
    w2t1_full[:, :T1] = (np.asarray(t1_w2, np.float32).T * W_SCALE).astype(FP8)

    iota = np.broadcast_to(
        np.arange(WH, dtype=np.float32)[None, :], (128, WH)
    ).copy()

    gi = np.where(tgt < C0, tgt, np.where(tgt < C1, C0, C0 + 1))
    rel0 = tgt - C0
    rel1 = tgt - C1

    # host-gathered target weight rows (bf16, matching device operand
    # precision), zeroed on cores that don't own the target's column shard
    t0_w2_bf = np.asarray(t0_w2, np.float32).astype(BF16)
    t1_w2_bf = np.asarray(t1_w2, np.float32).astype(BF16)

    def _gather_rows(tbl, row, own):
        g = tbl[np.clip(row, 0, tbl.shape[0] - 1)]
        g[~own] = 0
        return np.ascontiguousarray(
            g.reshape(MT, 128, tbl.shape[1]).transpose(1, 0, 2)
        )

    in_maps = []
    for i in range(NCORES):
        in_maps.append(
            {
                "inpT": inpT,
                "w1t0": w1t0,
                "w1t1": w1t1,
                "hwT": _tiled(hwT_full[:, i * WH : (i + 1) * WH]),
                "w2t0": _tiled(w2t0_full[:, i * W0 : (i + 1) * W0]),
                "w2t1": _tiled(w2t1_full[:, i * W1 : (i + 1) * W1]),
                "wg0": _gather_rows(t0_w2_bf, rel0, (rel0 // W0) == i),
                "wg1": _gather_rows(t1_w2_bf, rel1, (rel1 // W1) == i),
                "iota": iota,
                "rels": np.stack(
                    [
                        _pm((gi - i * WH).astype(np.float32)),
                        _pm((rel0 - i * W0).astype(np.float32)),
                        _pm((rel1 - i * W1).astype(np.float32)),
                    ],
                    axis=2,
                ).copy(),
            }
        )
    return in_maps, tgt


def combine(results, tgt):
    """results: list of per-core {'res': [128, MT, 6]} -> final [N] f32 NLL."""
    S = np.zeros((3, N), np.float64)
    T = np.zeros((3, N), np.float64)
    for r in results:
        res = np.asarray(r["res"], np.float64)
        for c in range(3):
            S[c] += _unpm(res[:, :, c])
            T[c] += _unpm(res[:, :, 3 + c])
    S[0] -= PAD_H  # zero-padded columns contribute exp(0)=1 each (core 7)
    S[1] -= PAD_0
    S[2] -= PAD_1

    in1 = (tgt >= C0) & (tgt < C1)
    in2 = tgt >= C1
    head_term = T[0] * HID_DESCALE - np.log(S[0])
    lp0 = T[1] - np.log(S[1])
    lp1 = T[2] - np.log(S[2])
    out = head_term + np.where(in1, lp0, 0.0) + np.where(in2, lp1, 0.0)
    return (-out).astype(np.float32)


def kernel(inp, tgt, head_w, t0_w1, t0_w2, t1_w1, t1_w2):
    global LAST_RESULT
    nc = _get_nc()
    in_maps, tgt64 = make_in_maps(inp, tgt, head_w, t0_w1, t0_w2, t1_w1, t1_w2)
    out = run_bass_kernel_spmd(
        nc, in_maps, core_ids=list(range(NCORES)), trace=TRACE
    )
    LAST_RESULT = out
    return combine(out.results, tgt64)



# revision 2
# speedup vs baseline: 1.2294x; 1.2294x over previous
"""Trainium2 Bass kernel for AdaptiveLogSoftmaxWithLoss (moe_routing).

Sharding: class dimensions are zero-padded and tensor-sharded across the 8
cores (head 4002->4096, tail0 16000, tail1 30257->30720), so every core runs
an identical SPMD program with 1/8 of the output classes.

Row compaction: the reference masks tail contributions, so only rows whose
target lies in cluster 1 (~620 of 2048) need the tail0 GEMM + logsumexp and
only cluster-2 rows (~1260) need tail1.  The host compacts those rows (padded
to 128-row tiles) and the device only computes the compacted tiles.  This
cuts the ACT-engine exp work (the bottleneck: 1 elem/cycle/partition) from
~13M to ~7.4M elements/core and the PE work similarly.

Per core the emission order is one interleaved stream that keeps ACT fed:
hidden0 first (smallest DMA prefix), then tail0 groups interleaved with head
groups and hidden1 chunks, then tail1 groups interleaved with the remaining
head groups.  PSUM: 3 two-bank slots (tail/hidden chunks) + 2 one-bank slots
(head groups) = 8 banks, so PE can run ~3 chunks ahead of ACT.

Host combine: sum exp-partials over cores, subtract the exact exp(0)=1
contribution of zero-padded columns, lse = log(sum); target logits come from
accum'd DVE dots (tails, host-gathered weight rows zeroed on non-owner
cores) and an (iota==rel)*logit pass (head); NLL assembled as in the
reference.  The host only shards, pads, quantizes, and combines [N]-vectors.
"""

import numpy as np
import ml_dtypes

import concourse.bass as bass
import concourse.bacc as bacc
import concourse.mybir as mybir
import concourse.tile as tile
from concourse.bass_utils import run_bass_kernel_spmd

BF16 = ml_dtypes.bfloat16
FP8 = ml_dtypes.float8_e4m3
H_SCALE = 8.0     # h cast to fp8 at 8x
W_SCALE = 64.0    # tail w2 cast to fp8 at 64x
IN_SCALE = 16.0   # inp cast to fp8 at 16x
W1_SCALE = 64.0   # w1 / head_w cast to fp8 at 64x
HID_DESCALE = 1.0 / (IN_SCALE * W1_SCALE)
DESCALE = 1.0 / (H_SCALE * W_SCALE)
NCORES = 8
N, D = 2048, 1024
H0, H1 = 512, 256
C0, C1 = 4000, 20000
HEAD = 4002        # 4000 shortlist + 2 cluster-logit columns
HEAD_PAD = 4096    # padded so 8 cores get 512 each
T0 = 16000
T1 = 30257
T1_PAD = 30720     # padded so 8 cores get 3840 each
WH, W0, W1 = HEAD_PAD // 8, T0 // 8, T1_PAD // 8     # 512, 2000, 3840
MT = N // 128                                        # 16 sample tiles
PAD_H = HEAD_PAD - HEAD   # 94 zero columns, all on core 7
PAD_1 = T1_PAD - T1       # 463 zero columns, all on core 7

# module-level knobs for test.py (harness never touches these)
TRACE = False
LAST_RESULT = None

_CACHED_NC = {}


def _chunks(total, step):
    out = []
    co = 0
    while co < total:
        out.append((co, min(step, total - co)))
        co += step
    return out


def _build_nc(P0, P1):
    N0, N1 = P0 * 128, P1 * 128
    nc = bacc.Bacc(None)
    BF = mybir.dt.bfloat16
    F8 = mybir.dt.float8e4
    F32 = mybir.dt.float32
    OP = mybir.AluOpType
    ACTF = mybir.ActivationFunctionType
    DR = mybir.MatmulPerfMode.DoubleRow
    KT = D // 128

    inpT_d = nc.dram_tensor("inpT", [128, KT, N], F8, kind="ExternalInput")
    inp0T_d = nc.dram_tensor("inp0T", [128, KT, N0], F8, kind="ExternalInput")
    inp1T_d = nc.dram_tensor("inp1T", [128, KT, N1], F8, kind="ExternalInput")
    w1t0_d = nc.dram_tensor("w1t0", [128, KT, H0], F8, kind="ExternalInput")
    w1t1_d = nc.dram_tensor("w1t1", [128, KT, H1], F8, kind="ExternalInput")
    hwT_d = nc.dram_tensor("hwT", [128, KT, WH], F8, kind="ExternalInput")
    w2t0_d = nc.dram_tensor("w2t0", [128, H0 // 128, W0], F8, kind="ExternalInput")
    w2t1_d = nc.dram_tensor("w2t1", [128, H1 // 128, W1], F8, kind="ExternalInput")
    wg0_d = nc.dram_tensor("wg0", [128, P0, H0], BF, kind="ExternalInput")
    wg1_d = nc.dram_tensor("wg1", [128, P1, H1], BF, kind="ExternalInput")
    iota_d = nc.dram_tensor("iota", [128, WH], F32, kind="ExternalInput")
    relh_d = nc.dram_tensor("relh", [128, MT], F32, kind="ExternalInput")
    resh_d = nc.dram_tensor("resh", [128, MT, 2], F32, kind="ExternalOutput")
    res0_d = nc.dram_tensor("res0", [128, P0, 3], F32, kind="ExternalOutput")
    res1_d = nc.dram_tensor("res1", [128, P1, 5], F32, kind="ExternalOutput")

    with tile.TileContext(nc) as tc:
        with (
            tc.tile_pool(name="const", bufs=1) as cp,
            tc.tile_pool(name="work", bufs=3) as wp,
            tc.tile_pool(name="psumB", bufs=3, space="PSUM") as bigp,
            tc.tile_pool(name="psumS", bufs=2, space="PSUM") as smlp,
        ):
            inpT = cp.tile([128, KT, N], F8)
            inp0T = cp.tile([128, KT, N0], F8)
            inp1T = cp.tile([128, KT, N1], F8)
            w1t0 = cp.tile([128, KT, H0], F8)
            w1t1 = cp.tile([128, KT, H1], F8)
            hwT = cp.tile([128, KT, WH], F8)
            w2t0 = cp.tile([128, H0 // 128, W0], F8)
            w2t1 = cp.tile([128, H1 // 128, W1], F8)
            wg0 = cp.tile([128, P0, H0], BF)
            wg1 = cp.tile([128, P1, H1], BF)
            iota = cp.tile([128, WH], F32)
            relh = cp.tile([128, MT], F32)
            h0T = cp.tile([128, H0 // 128, N0], BF)
            h1T = cp.tile([128, H1 // 128, N1], BF)
            h0T8 = cp.tile([128, H0 // 128, N0], F8)
            h1T8 = cp.tile([128, H1 // 128, N1], F8)
            h0n = cp.tile([128, P0, H0], BF)
            h1n = cp.tile([128, P1, H1], BF)
            resh = cp.tile([128, MT, 2], F32)
            res0 = cp.tile([128, P0, 3], F32)
            res1 = cp.tile([128, P1, 5], F32)

            # loads ordered by first use: hidden0 path, tail0 weights, head,
            # hidden1 path, tail1 weights
            nc.sync.dma_start(iota[:], iota_d[:])
            nc.sync.dma_start(relh[:], relh_d[:])
            nc.sync.dma_start(inp0T[:], inp0T_d[:])
            nc.sync.dma_start(w1t0[:], w1t0_d[:])
            nc.sync.dma_start(w2t0[:], w2t0_d[:])
            nc.sync.dma_start(wg0[:], wg0_d[:])
            for kt in range(KT):
                nc.sync.dma_start(inpT[:, kt], inpT_d[:, kt])
            nc.sync.dma_start(hwT[:], hwT_d[:])
            nc.sync.dma_start(inp1T[:], inp1T_d[:])
            nc.sync.dma_start(w1t1[:], w1t1_d[:])
            nc.sync.dma_start(w2t1[:], w2t1_d[:])
            nc.sync.dma_start(wg1[:], wg1_d[:])

            # tiny first ACT op so the ~2.7us exp table load overlaps the
            # initial DMA wait instead of delaying the first real exp
            warm = wp.tile([128, 8], BF, tag="warm")
            nc.scalar.activation(warm[:], iota[:, 0:8], ACTF.Exp)

            def hid_chunk(inT, w1, hT, hT8, mh, co, cw):
                # h[mh*128:(mh+1)*128, co:co+cw] for the compacted samples
                ps = bigp.tile([128, 1024], F32, tag="big", name="ps")
                for cc, sub in _chunks(cw, 512):
                    for kt in range(0, KT, 2):
                        nc.tensor.matmul(
                            ps[:, cc : cc + sub],
                            w1[:, kt : kt + 2, mh * 128 : (mh + 1) * 128],
                            inT[:, kt : kt + 2, co + cc : co + cc + sub],
                            start=(kt == 0),
                            stop=(kt + 2 >= KT),
                            perf_mode=DR,
                        )
                nc.vector.tensor_scalar_mul(
                    hT[:, mh, co : co + cw], ps[:, :cw], HID_DESCALE
                )
                nc.vector.tensor_scalar_mul(
                    hT8[:, mh, co : co + cw], hT[:, mh, co : co + cw], H_SCALE
                )

            def head_job(m):
                ms = slice(m * 128, (m + 1) * 128)
                ps = smlp.tile([128, 512], F32, tag="sml", name="ps")
                for kt in range(0, KT, 2):
                    nc.tensor.matmul(
                        ps[:],
                        inpT[:, kt : kt + 2, ms],
                        hwT[:, kt : kt + 2, :],
                        start=(kt == 0),
                        stop=(kt + 2 >= KT),
                        perf_mode=DR,
                    )
                sc_e = wp.tile([128, 512], BF, tag="sc_eh")
                nc.scalar.activation(
                    sc_e[:],
                    ps[:],
                    ACTF.Exp,
                    scale=HID_DESCALE,
                    accum_out=resh[:, m, 0:1],
                )
                sc_t = wp.tile([128, 512], BF, tag="sc_t")
                nc.vector.scalar_tensor_tensor(
                    out=sc_t[:],
                    in0=iota[:],
                    scalar=relh[:, m : m + 1],
                    in1=ps[:],
                    op0=OP.is_equal,
                    op1=OP.mult,
                    accum_out=resh[:, m, 1:2],
                )

            def tail_chunk(lhsT, w2, kdim, m, co, cw, s_ap):
                ms = slice(m * 128, (m + 1) * 128)
                ps = bigp.tile([128, 1024], F32, tag="big", name="ps")
                nsub = kdim // 128
                for cc, sub in _chunks(cw, 512):
                    for kt in range(0, nsub, 2):
                        nc.tensor.matmul(
                            ps[:, cc : cc + sub],
                            lhsT[:, kt : kt + 2, ms],
                            w2[:, kt : kt + 2, co + cc : co + cc + sub],
                            start=(kt == 0),
                            stop=(kt + 2 >= nsub),
                            perf_mode=DR,
                        )
                sc_e = wp.tile([128, 1024], BF, tag="sc_e")
                nc.scalar.activation(
                    sc_e[:, :cw], ps[:, :cw], ACTF.Exp, scale=DESCALE,
                    accum_out=s_ap,
                )

            def dot(hn, wg, hdim, m, t_ap):
                sc_d = wp.tile([128, H0], BF, tag="sc_d")
                nc.vector.scalar_tensor_tensor(
                    out=sc_d[:, :hdim],
                    in0=hn[:, m, :],
                    scalar=1.0,
                    in1=wg[:, m, :],
                    op0=OP.mult,
                    op1=OP.mult,
                    accum_out=t_ap,
                )

            def t0_job(m):
                for ci, (co, cw) in enumerate(_chunks(W0, 1024)):
                    tail_chunk(h0T8, w2t0, H0, m, co, cw, res0[:, m, ci : ci + 1])
                dot(h0n, wg0, H0, m, res0[:, m, 2:3])

            def t1_job(m):
                for ci, (co, cw) in enumerate(_chunks(W1, 1024)):
                    tail_chunk(h1T8, w2t1, H1, m, co, cw, res1[:, m, ci : ci + 1])
                dot(h1n, wg1, H1, m, res1[:, m, 4:5])

            with nc.named_scope("hidden0"):
                for mh in range(H0 // 128):
                    hid_chunk(inp0T, w1t0, h0T, h0T8, mh, 0, N0)
                for kt in range(H0 // 128):
                    nc.sync.dma_start_transpose(
                        h0n[:, :, kt * 128 : (kt + 1) * 128], h0T[:, kt, :]
                    )

            # interleave: tail0 groups + head groups + hidden1 chunks
            hid1_jobs = [
                (mh, co, cw)
                for mh in range(H1 // 128)
                for co, cw in _chunks(N1, 1024)
            ]
            heads = list(range(MT))
            with nc.named_scope("tail0_head_hidden1"):
                for m in range(P0):
                    t0_job(m)
                    if heads:
                        head_job(heads.pop(0))
                    for _ in range(-(-len(hid1_jobs) * 1 // P0)):
                        if hid1_jobs:
                            mh, co, cw = hid1_jobs.pop(0)
                            hid_chunk(inp1T, w1t1, h1T, h1T8, mh, co, cw)
                for kt in range(H1 // 128):
                    nc.sync.dma_start_transpose(
                        h1n[:, :, kt * 128 : (kt + 1) * 128], h1T[:, kt, :]
                    )

            with nc.named_scope("tail1_head"):
                for m in range(P1):
                    t1_job(m)
                    for _ in range(2):
                        if heads:
                            head_job(heads.pop(0))
                while heads:
                    head_job(heads.pop(0))

            nc.sync.dma_start(resh_d[:], resh[:])
            nc.sync.dma_start(res0_d[:], res0[:])
            nc.sync.dma_start(res1_d[:], res1[:])

    nc.finalize()
    return nc


def _get_nc(P0, P1):
    key = (P0, P1)
    if key not in _CACHED_NC:
        _CACHED_NC[key] = _build_nc(P0, P1)
    return _CACHED_NC[key]


def _tiled(a2d):
    """[K, F] (K multiple of 128) -> contiguous [128, K//128, F]."""
    K, F = a2d.shape
    return np.ascontiguousarray(
        a2d.reshape(K // 128, 128, F).transpose(1, 0, 2)
    )


def _pm(vec):
    """[n] (multiple of 128) -> [128, n//128] with [p, m] = vec[m*128+p]."""
    n = vec.shape[0]
    return np.ascontiguousarray(vec.reshape(n // 128, 128).T)


def _unpm(a):
    """[128, m] -> [m*128]."""
    return np.ascontiguousarray(a.T).reshape(-1)


def make_in_maps(inp, tgt, head_w, t0_w1, t0_w2, t1_w1, t1_w2):
    inp = np.asarray(inp, dtype=np.float32)
    tgt = np.asarray(tgt).astype(np.int64)

    in1 = (tgt >= C0) & (tgt < C1)
    in2 = tgt >= C1
    idx0 = np.where(in1)[0]
    idx1 = np.where(in2)[0]
    n0, n1 = len(idx0), len(idx1)
    P0 = max(1, -(-n0 // 128))
    P1 = max(1, -(-n1 // 128))
    idx0p = np.concatenate([idx0, np.zeros(P0 * 128 - n0, np.int64)])
    idx1p = np.concatenate([idx1, np.zeros(P1 * 128 - n1, np.int64)])

    inpT = _tiled((inp.T * IN_SCALE).astype(FP8))
    inp0T = _tiled((inp[idx0p].T * IN_SCALE).astype(FP8))
    inp1T = _tiled((inp[idx1p].T * IN_SCALE).astype(FP8))
    w1t0 = _tiled((np.asarray(t0_w1, np.float32).T * W1_SCALE).astype(FP8))
    w1t1 = _tiled((np.asarray(t1_w1, np.float32).T * W1_SCALE).astype(FP8))

    hwT_full = np.zeros((D, HEAD_PAD), FP8)
    hwT_full[:, :HEAD] = (np.asarray(head_w, np.float32).T * W1_SCALE).astype(FP8)
    w2t0_full = (np.asarray(t0_w2, np.float32).T * W_SCALE).astype(FP8)
    w2t1_full = np.zeros((H1, T1_PAD), FP8)
    w2t1_full[:, :T1] = (np.asarray(t1_w2, np.float32).T * W_SCALE).astype(FP8)

    iota = np.broadcast_to(
        np.arange(WH, dtype=np.float32)[None, :], (128, WH)
    ).copy()

    gi = np.where(tgt < C0, tgt, np.where(tgt < C1, C0, C0 + 1))
    rel0 = tgt[idx0p] - C0
    rel1 = tgt[idx1p] - C1

    # host-gathered target weight rows (bf16, matching device operand
    # precision), zeroed on cores that don't own the target's column shard
    t0_w2_bf = np.asarray(t0_w2, np.float32).astype(BF16)
    t1_w2_bf = np.asarray(t1_w2, np.float32).astype(BF16)

    def _gather_rows(tbl, row, own):
        g = tbl[np.clip(row, 0, tbl.shape[0] - 1)]
        g[~own] = 0
        return np.ascontiguousarray(
            g.reshape(-1, 128, tbl.shape[1]).transpose(1, 0, 2)
        )

    in_maps = []
    for i in range(NCORES):
        in_maps.append(
            {
                "inpT": inpT,
                "inp0T": inp0T,
                "inp1T": inp1T,
                "w1t0": w1t0,
                "w1t1": w1t1,
                "hwT": _tiled(hwT_full[:, i * WH : (i + 1) * WH]),
                "w2t0": _tiled(w2t0_full[:, i * W0 : (i + 1) * W0]),
                "w2t1": _tiled(w2t1_full[:, i * W1 : (i + 1) * W1]),
                "wg0": _gather_rows(t0_w2_bf, rel0, (rel0 // W0) == i),
                "wg1": _gather_rows(t1_w2_bf, rel1, (rel1 // W1) == i),
                "iota": iota,
                "relh": _pm((gi - i * WH).astype(np.float32)),
            }
        )
    return in_maps, tgt, (idx0, idx1, n0, n1, P0, P1)


def combine(results, tgt, meta):
    """per-core {'resh','res0','res1'} partials -> final [N] f32 NLL."""
    idx0, idx1, n0, n1, P0, P1 = meta
    Sh = np.zeros((128, MT), np.float64)
    Th = np.zeros((128, MT), np.float64)
    S0 = np.zeros((128, P0), np.float64)
    T0s = np.zeros((128, P0), np.float64)
    S1 = np.zeros((128, P1), np.float64)
    T1s = np.zeros((128, P1), np.float64)
    for r in results:
        resh = np.asarray(r["resh"], np.float64)
        res0 = np.asarray(r["res0"], np.float64)
        res1 = np.asarray(r["res1"], np.float64)
        Sh += resh[:, :, 0]
        Th += resh[:, :, 1]
        S0 += res0[:, :, 0:2].sum(axis=2)
        T0s += res0[:, :, 2]
        S1 += res1[:, :, 0:4].sum(axis=2)
        T1s += res1[:, :, 4]

    # zero-padded columns contribute exp(0)=1 each (core 7)
    head_term = _unpm(Th) * HID_DESCALE - np.log(_unpm(Sh) - PAD_H)
    lp0 = _unpm(T0s) - np.log(_unpm(S0))
    lp1 = _unpm(T1s) - np.log(_unpm(S1) - PAD_1)

    out = head_term
    out[idx0] += lp0[:n0]
    out[idx1] += lp1[:n1]
    return (-out).astype(np.float32)


def kernel(inp, tgt, head_w, t0_w1, t0_w2, t1_w1, t1_w2):
    global LAST_RESULT
    in_maps, tgt64, meta = make_in_maps(
        inp, tgt, head_w, t0_w1, t0_w2, t1_w1, t1_w2
    )
    nc = _get_nc(meta[4], meta[5])
    out = run_bass_kernel_spmd(
        nc, in_maps, core_ids=list(range(NCORES)), trace=TRACE
    )
    LAST_RESULT = out
    return combine(out.results, tgt64, meta)


# revision 5
# speedup vs baseline: 1.2890x; 1.0485x over previous
"""Trainium2 Bass kernel for AdaptiveLogSoftmaxWithLoss (moe_routing).

Work split across the 8 cores (all fp8 DoubleRow GEMMs at 2 MACs/cell/cycle):
  - tails tensor-sharded by class columns (t0: 2000/core, t1: 3840/core,
    zero-padded on core 7) over host-compacted member rows only (the
    reference masks non-member rows; ~620 rows for t0, ~1260 for t1).
  - head sharded over (sample-tile quads x column halves): core pair
    (2j, 2j+1) owns sample tiles 4j..4j+3; even core takes head cols
    0:2048, odd core 2048:4096.  Fewer, larger ACT chunks than pure
    column sharding (the ACT fixed cost + accumulator read is ~1us/chunk).
  - hidden projections (h0, h1) replicated per core over compacted rows,
    fused descale+fp8 cast in one DVE op.

Target logits come from per-tile diagonal GEMMs: lhsT = the same
inpT/hidden fp8 slices, rhs = host-gathered fp8 weight rows of each
sample's target (zeroed on non-owner cores); a DVE (iota==p)*psum pass
extracts the diagonal with a dve-accumulator read.

Per-row sum-exp: one ACT exp+accum per PSUM chunk (2 x [128,2048] PSUM
slots rotate; PE fills one while ACT/DVE drain the other).  The tail1
B-chunks [1792] instead use a Schraudolph exp on the DVE (int32(x*K1+B)
bitcast to f32, mean-unbiased C) to balance the scalar and vector
engines; the host corrects core 7's zero-padded columns by the exact
approx-exp(0) value.

Host combine: sum partials over cores, lse = log(sum), NLL as in the
reference.  The host only shards, pads, quantizes, gathers and combines.
"""

import numpy as np
import ml_dtypes

import concourse.bass as bass
import concourse.bacc as bacc
import concourse.mybir as mybir
import concourse.tile as tile
from concourse.bass_utils import run_bass_kernel_spmd

BF16 = ml_dtypes.bfloat16
FP8 = ml_dtypes.float8_e4m3
H_SCALE = 8.0     # h cast to fp8 at 8x
W_SCALE = 64.0    # tail w2 cast to fp8 at 64x
IN_SCALE = 16.0   # inp cast to fp8 at 16x
W1_SCALE = 64.0   # w1 / head_w cast to fp8 at 64x
HID_DESCALE = 1.0 / (IN_SCALE * W1_SCALE)
DESCALE = 1.0 / (H_SCALE * W_SCALE)
NCORES = 8
N, D = 2048, 1024
H0, H1 = 512, 256
C0, C1 = 4000, 20000
HEAD = 4002
HEAD_PAD = 4096
T0 = 16000
T1 = 30257
T1_PAD = 30720
WH, W0, W1 = HEAD_PAD // 2, T0 // 8, T1_PAD // 8     # 2048, 2000, 3840
MT = N // 128                                        # 16 sample tiles
PAD_H = HEAD_PAD - HEAD   # 94 zero cols, odd cores' half
PAD_1 = T1_PAD - T1       # 463 zero cols, core 7 (all in the B-chunk)

# Schraudolph exp: exp(ps*DESCALE) ~= bitcast_f32(int32(ps*SCH_K1 + SCH_B))
SCH_C = 473120.0          # tuned for zero mean relative bias
SCH_K1 = float(np.float32((2 ** 23) / np.log(2) / 512.0))
SCH_B = float(np.float32(1065353216.0 - SCH_C))
SCH_E0 = float(np.int32(np.float32(SCH_B)).view(np.float32))  # approx exp(0)

TRACE = False
LAST_RESULT = None

_CACHED_NC = {}


def _chunks(total, step, off=0):
    out = []
    co = 0
    while co < total:
        out.append((off + co, min(step, total - co)))
        co += step
    return out


def _build_nc(P0, P1):
    N0, N1 = P0 * 128, P1 * 128
    nc = bacc.Bacc(None)
    BF = mybir.dt.bfloat16
    F8 = mybir.dt.float8e4
    F32 = mybir.dt.float32
    I32 = mybir.dt.int32
    OP = mybir.AluOpType
    ACTF = mybir.ActivationFunctionType
    DR = mybir.MatmulPerfMode.DoubleRow
    KT = D // 128

    pidx_d = nc.dram_tensor("pidx", [128, 1], F32, kind="ExternalInput")
    iota_d = nc.dram_tensor("iota", [128, 128], F32, kind="ExternalInput")
    inp0T_d = nc.dram_tensor("inp0T", [128, KT, N0], F8, kind="ExternalInput")
    w1t0_d = nc.dram_tensor("w1t0", [128, KT, H0], F8, kind="ExternalInput")
    w2t0_d = nc.dram_tensor("w2t0", [128, H0 // 128, W0], F8, kind="ExternalInput")
    wgT0_d = nc.dram_tensor("wgT0", [128, H0 // 128, N0], F8, kind="ExternalInput")
    inp1T_d = nc.dram_tensor("inp1T", [128, KT, N1], F8, kind="ExternalInput")
    w1t1_d = nc.dram_tensor("w1t1", [128, KT, H1], F8, kind="ExternalInput")
    w2t1_d = nc.dram_tensor("w2t1", [128, H1 // 128, W1], F8, kind="ExternalInput")
    wgT1_d = nc.dram_tensor("wgT1", [128, H1 // 128, N1], F8, kind="ExternalInput")
    inpH_d = nc.dram_tensor("inpH", [128, KT, 512], F8, kind="ExternalInput")
    hwT_d = nc.dram_tensor("hwT", [128, KT, WH], F8, kind="ExternalInput")
    wgH_d = nc.dram_tensor("wgH", [128, KT, 512], F8, kind="ExternalInput")
    resh_d = nc.dram_tensor("resh", [128, 4, 2], F32, kind="ExternalOutput")
    res0_d = nc.dram_tensor("res0", [128, P0, 2], F32, kind="ExternalOutput")
    res1_d = nc.dram_tensor("res1", [128, P1, 3], F32, kind="ExternalOutput")

    with tile.TileContext(nc) as tc:
        with (
            tc.tile_pool(name="const", bufs=1) as cp,
            tc.tile_pool(name="work", bufs=3) as wp,
            tc.tile_pool(name="psum", bufs=2, space="PSUM") as bigp,
        ):
            pidx = cp.tile([128, 1], F32)
            iota = cp.tile([128, 128], F32)
            inp0T = cp.tile([128, KT, N0], F8)
            w1t0 = cp.tile([128, KT, H0], F8)
            w2t0 = cp.tile([128, H0 // 128, W0], F8)
            wgT0 = cp.tile([128, H0 // 128, N0], F8)
            inp1T = cp.tile([128, KT, N1], F8)
            w1t1 = cp.tile([128, KT, H1], F8)
            w2t1 = cp.tile([128, H1 // 128, W1], F8)
            wgT1 = cp.tile([128, H1 // 128, N1], F8)
            inpH = cp.tile([128, KT, 512], F8)
            hwT = cp.tile([128, KT, WH], F8)
            wgH = cp.tile([128, KT, 512], F8)
            h0T8 = cp.tile([128, H0 // 128, N0], F8)
            h1T8 = cp.tile([128, H1 // 128, N1], F8)
            resh = cp.tile([128, 4, 2], F32)
            res0 = cp.tile([128, P0, 2], F32)
            res1 = cp.tile([128, P1, 3], F32)

            # loads in first-use order; k-pair interleave for the hidden0
            # path so its kt-major GEMM can start on the first pair
            nc.sync.dma_start(pidx[:], pidx_d[:])
            nc.sync.dma_start(iota[:], iota_d[:])
            for kt in range(0, KT, 2):
                nc.sync.dma_start(inp0T[:, kt : kt + 2], inp0T_d[:, kt : kt + 2])
                nc.sync.dma_start(w1t0[:, kt : kt + 2], w1t0_d[:, kt : kt + 2])
            nc.sync.dma_start(w2t0[:, 0:2], w2t0_d[:, 0:2])
            nc.sync.dma_start(w2t0[:, 2:4], w2t0_d[:, 2:4])
            nc.sync.dma_start(wgT0[:], wgT0_d[:])
            nc.sync.dma_start(inp1T[:], inp1T_d[:])
            nc.sync.dma_start(w1t1[:], w1t1_d[:])
            nc.sync.dma_start(w2t1[:], w2t1_d[:])
            nc.sync.dma_start(wgT1[:], wgT1_d[:])
            nc.sync.dma_start(inpH[:], inpH_d[:])
            for kt in range(0, KT, 2):
                nc.sync.dma_start(hwT[:, kt : kt + 2], hwT_d[:, kt : kt + 2])
            nc.sync.dma_start(wgH[:], wgH_d[:])

            # preload the exp table during the DMA ramp
            warm = wp.tile([128, 1], BF, tag="warm")
            nc.scalar.activation(warm[:], pidx[:], ACTF.Exp)

            # warm the PE HAM clock gate during the DMA ramp (fp32 matmuls
            # on the tiny iota tile); the dummy DVE read frees the slot
            psw = bigp.tile([128, 2048], F32, tag="big", name="ps")
            for _ in range(40):
                nc.tensor.matmul(psw[:, :128], iota[:], iota[:], start=True, stop=True)
            wsink = wp.tile([128, 1], F32, tag="wsink")
            nc.vector.tensor_scalar_mul(wsink[:], psw[:, 0:1], 0.0)

            def mm_block(ps, fchunks, nkt, lhsT_fn, rhs_fn):
                kts = list(range(0, nkt, 2))
                for ki, kt in enumerate(kts):
                    for co, cw in fchunks:
                        nc.tensor.matmul(
                            ps[:, co : co + cw],
                            lhsT_fn(kt),
                            rhs_fn(kt, co, cw),
                            start=(ki == 0),
                            stop=(ki == len(kts) - 1),
                            perf_mode=DR,
                        )

            def hid_job(inT, w1, hT8, mh, width):
                ps = bigp.tile([128, 2048], F32, tag="big", name="ps")
                mm_block(
                    ps, _chunks(width, 512), KT,
                    lambda kt: w1[:, kt : kt + 2, mh * 128 : (mh + 1) * 128],
                    lambda kt, co, cw: inT[:, kt : kt + 2, co : co + cw],
                )
                nc.vector.tensor_scalar_mul(
                    hT8[:, mh, :], ps[:, :width], HID_DESCALE * H_SCALE
                )

            def exp_job(ps, cw, scale, s_ap):
                sc_e = wp.tile([128, 2048], BF, tag="sc_e")
                nc.scalar.activation(
                    sc_e[:, :cw], ps[:, :cw], ACTF.Exp, scale=scale, accum_out=s_ap
                )

            def schraud_job(ps, cw, s_ap):
                e32 = wp.tile([128, 2048], I32, tag="e32")
                nc.vector.tensor_scalar(
                    out=e32[:, :cw], in0=ps[:, :cw],
                    scalar1=SCH_K1, scalar2=SCH_B,
                    op0=OP.mult, op1=OP.add,
                )
                sc2 = wp.tile([128, 2048], BF, tag="sc2")
                nc.vector.tensor_scalar(
                    out=sc2[:, :cw], in0=e32[:, :cw].bitcast(F32),
                    scalar1=1.0, scalar2=0.0, op0=OP.mult, op1=OP.add,
                    accum_out=s_ap,
                )

            def diag_job(lhsT_fn, wgT, nkt, ms, t_ap):
                ps = bigp.tile([128, 2048], F32, tag="big", name="ps")
                mm_block(
                    ps, [(0, 128)], nkt,
                    lhsT_fn,
                    lambda kt, co, cw: wgT[:, kt : kt + 2, ms],
                )
                sc_g = wp.tile([128, 128], BF, tag="sc_g")
                nc.vector.scalar_tensor_tensor(
                    out=sc_g[:],
                    in0=iota[:],
                    scalar=pidx[:, 0:1],
                    in1=ps[:, :128],
                    op0=OP.is_equal,
                    op1=OP.mult,
                    accum_out=t_ap,
                )

            def t0_job(m):
                ms = slice(m * 128, (m + 1) * 128)
                ps = bigp.tile([128, 2048], F32, tag="big", name="ps")
                mm_block(
                    ps, _chunks(W0, 512), H0 // 128,
                    lambda kt: h0T8[:, kt : kt + 2, ms],
                    lambda kt, co, cw: w2t0[:, kt : kt + 2, co : co + cw],
                )
                exp_job(ps, W0, DESCALE, res0[:, m, 0:1])
                diag_job(
                    lambda kt: h0T8[:, kt : kt + 2, ms], wgT0, H0 // 128, ms,
                    res0[:, m, 1:2],
                )

            def t1_job(m):
                ms = slice(m * 128, (m + 1) * 128)
                psA = bigp.tile([128, 2048], F32, tag="big", name="ps")
                mm_block(
                    psA, _chunks(2048, 512), H1 // 128,
                    lambda kt: h1T8[:, kt : kt + 2, ms],
                    lambda kt, co, cw: w2t1[:, kt : kt + 2, co : co + cw],
                )
                exp_job(psA, 2048, DESCALE, res1[:, m, 0:1])
                psB = bigp.tile([128, 2048], F32, tag="big", name="ps")
                mm_block(
                    psB, _chunks(W1 - 2048, 512), H1 // 128,
                    lambda kt: h1T8[:, kt : kt + 2, ms],
                    lambda kt, co, cw: w2t1[:, kt : kt + 2, 2048 + co : 2048 + co + cw],
                )
                schraud_job(psB, W1 - 2048, res1[:, m, 1:2])
                diag_job(
                    lambda kt: h1T8[:, kt : kt + 2, ms], wgT1, H1 // 128, ms,
                    res1[:, m, 2:3],
                )

            def head_job(lt):
                ls = slice(lt * 128, (lt + 1) * 128)
                ps = bigp.tile([128, 2048], F32, tag="big", name="ps")
                mm_block(
                    ps, _chunks(WH, 512), KT,
                    lambda kt: inpH[:, kt : kt + 2, ls],
                    lambda kt, co, cw: hwT[:, kt : kt + 2, co : co + cw],
                )
                exp_job(ps, WH, HID_DESCALE, resh[:, lt, 0:1])
                diag_job(
                    lambda kt: inpH[:, kt : kt + 2, ls], wgH, KT, ls,
                    resh[:, lt, 1:2],
                )

            with nc.named_scope("hid0_t0"):
                for mh in range(H0 // 128):
                    hid_job(inp0T, w1t0, h0T8, mh, N0)
                for m in range(P0):
                    t0_job(m)
            with nc.named_scope("hid1_t1_head"):
                for mh in range(H1 // 128):
                    hid_job(inp1T, w1t1, h1T8, mh, N1)
                nhead = 4
                for m in range(P1):
                    t1_job(m)
                    if m >= P1 - nhead - 1 and nhead > 0:
                        nhead -= 1
                        head_job(3 - nhead)
                while nhead > 0:
                    nhead -= 1
                    head_job(3 - nhead)

            nc.sync.dma_start(resh_d[:], resh[:])
            nc.sync.dma_start(res0_d[:], res0[:])
            nc.sync.dma_start(res1_d[:], res1[:])

    nc.finalize()
    return nc


def _get_nc(P0, P1):
    key = (P0, P1)
    if key not in _CACHED_NC:
        _CACHED_NC[key] = _build_nc(P0, P1)
    return _CACHED_NC[key]


def _tiled(a2d):
    """[K, F] (K multiple of 128) -> contiguous [128, K//128, F]."""
    K, F = a2d.shape
    return np.ascontiguousarray(
        a2d.reshape(K // 128, 128, F).transpose(1, 0, 2)
    )


def _unpm(a):
    """[128, m] -> [m*128]."""
    return np.ascontiguousarray(a.T).reshape(-1)


def make_in_maps(inp, tgt, head_w, t0_w1, t0_w2, t1_w1, t1_w2):
    inp = np.asarray(inp, dtype=np.float32)
    tgt = np.asarray(tgt).astype(np.int64)

    in1 = (tgt >= C0) & (tgt < C1)
    in2 = tgt >= C1
    idx0 = np.where(in1)[0]
    idx1 = np.where(in2)[0]
    n0, n1 = len(idx0), len(idx1)
    P0 = max(1, -(-n0 // 128))
    P1 = max(1, -(-n1 // 128))
    idx0p = np.concatenate([idx0, np.zeros(P0 * 128 - n0, np.int64)])
    idx1p = np.concatenate([idx1, np.zeros(P1 * 128 - n1, np.int64)])

    inpT_s = (inp.T * IN_SCALE).astype(FP8)           # [D, N]
    inp0T = _tiled(np.ascontiguousarray(inpT_s[:, idx0p]))
    inp1T = _tiled(np.ascontiguousarray(inpT_s[:, idx1p]))
    w1t0 = _tiled((np.asarray(t0_w1, np.float32).T * W1_SCALE).astype(FP8))
    w1t1 = _tiled((np.asarray(t1_w1, np.float32).T * W1_SCALE).astype(FP8))

    hwT_full = np.zeros((D, HEAD_PAD), FP8)
    hwT_full[:, :HEAD] = (np.asarray(head_w, np.float32).T * W1_SCALE).astype(FP8)
    w2t0_full = (np.asarray(t0_w2, np.float32).T * W_SCALE).astype(FP8)
    w2t1_full = np.zeros((H1, T1_PAD), FP8)
    w2t1_full[:, :T1] = (np.asarray(t1_w2, np.float32).T * W_SCALE).astype(FP8)

    gi = np.where(tgt < C0, tgt, np.where(tgt < C1, C0, C0 + 1))
    rel0 = tgt[idx0p] - C0
    rel1 = tgt[idx1p] - C1

    def _gathT(full, rel, own):
        # [K, osz] -> gathered [K, nrows], zeroed on non-owner cores
        g = np.ascontiguousarray(full[:, np.clip(rel, 0, full.shape[1] - 1)])
        g[:, ~own] = 0
        return _tiled(g)

    iota = np.broadcast_to(
        np.arange(128, dtype=np.float32)[None, :], (128, 128)
    ).copy()
    pidx = np.arange(128, dtype=np.float32)[:, None].copy()

    in_maps = []
    for i in range(NCORES):
        j, h = i // 2, i % 2
        smp = slice(j * 512, (j + 1) * 512)
        gih = gi[smp]
        wgH_full = np.ascontiguousarray(hwT_full[:, gih])
        if h == 1:
            wgH_full = np.zeros_like(wgH_full)
        in_maps.append(
            {
                "pidx": pidx,
                "iota": iota,
                "inp0T": inp0T,
                "w1t0": w1t0,
                "w2t0": _tiled(w2t0_full[:, i * W0 : (i + 1) * W0]),
                "wgT0": _gathT(w2t0_full, rel0, (rel0 // W0) == i),
                "inp1T": inp1T,
                "w1t1": w1t1,
                "w2t1": _tiled(w2t1_full[:, i * W1 : (i + 1) * W1]),
                "wgT1": _gathT(w2t1_full, rel1, (rel1 // W1) == i),
                "inpH": _tiled(np.ascontiguousarray(inpT_s[:, smp])),
                "hwT": _tiled(hwT_full[:, h * WH : (h + 1) * WH]),
                "wgH": _tiled(wgH_full),
            }
        )
    return in_maps, tgt, (idx0, idx1, n0, n1, P0, P1)


def combine(results, tgt, meta):
    """per-core {'resh','res0','res1'} partials -> final [N] f32 NLL."""
    idx0, idx1, n0, n1, P0, P1 = meta
    Sh = np.zeros((128, MT), np.float64)
    Th = np.zeros((128, MT), np.float64)
    S0 = np.zeros((128, P0), np.float64)
    T0s = np.zeros((128, P0), np.float64)
    S1 = np.zeros((128, P1), np.float64)
    T1s = np.zeros((128, P1), np.float64)
    for i, r in enumerate(results):
        j = i // 2
        resh = np.asarray(r["resh"], np.float64)
        res0 = np.asarray(r["res0"], np.float64)
        res1 = np.asarray(r["res1"], np.float64)
        Sh[:, 4 * j : 4 * j + 4] += resh[:, :, 0]
        Th[:, 4 * j : 4 * j + 4] += resh[:, :, 1]
        S0 += res0[:, :, 0]
        T0s += res0[:, :, 1]
        S1 += res1[:, :, 0] + res1[:, :, 1]
        T1s += res1[:, :, 2]

    # zero-padded cols: head pad on odd cores' halves (exp(0)=1 each);
    # tail1 pad all in core 7's Schraudolph B-chunk (approx exp(0)=SCH_E0)
    head_term = _unpm(Th) * HID_DESCALE - np.log(_unpm(Sh) - PAD_H)
    lp0 = _unpm(T0s) * DESCALE - np.log(_unpm(S0))
    lp1 = _unpm(T1s) * DESCALE - np.log(_unpm(S1) - PAD_1 * SCH_E0)

    out = head_term
    out[idx0] += lp0[:n0]
    out[idx1] += lp1[:n1]
    return (-out).astype(np.float32)


def kernel(inp, tgt, head_w, t0_w1, t0_w2, t1_w1, t1_w2):
    global LAST_RESULT
    in_maps, tgt64, meta = make_in_maps(
        inp, tgt, head_w, t0_w1, t0_w2, t1_w1, t1_w2
    )
    nc = _get_nc(meta[4], meta[5])
    out = run_bass_kernel_spmd(
        nc, in_maps, core_ids=list(range(NCORES)), trace=TRACE
    )
    LAST_RESULT = out
    return combine(out.results, tgt64, meta)
